# revision 1
# baseline (speedup 1.0000x reference)
"""Segment-mean (CGCNN crystal pooling) Bass kernel for 8 Trainium2 NeuronCores.

Reference computes, for sorted segment_ids over 1M atoms with 128 features:
    out[s] = sum(atom_fea[segment_ids == s]) / max(count(s), 1)   for s in [0, 16384)

Strategy (data-parallel over crystals, no cross-device communication):
  - Core c owns segments [2048*c, 2048*(c+1)) = 32 groups of 64 segments.
  - Host pads each 64-segment group's atoms to a uniform budget B = T*128 and
    lays them out partition-major: fea[g] is [128, T*128] where column block t
    holds atom tile t ([128 atoms in partitions] x [128 features]).
  - Device, per group: accumulate PSUM[128 fea, 64 seg] over T matmuls
    (lhsT = fea tile [128 atoms, 128 fea], rhs = one-hot [128 atoms, 64 segs]).
    The one-hot is built on DVE with one tensor_scalar is_equal against a
    constant iota row, using the per-atom relative segment id as the
    per-partition scalar. Padding atoms carry id -1 (never matches) and zero
    features. Evict PSUM with one elementwise multiply by 1/count (broadcast
    across partitions), then DMA [128 fea, 64 seg] to DRAM.
  - Host reassembles: transpose each [fea, seg] group slab to [seg, fea].
"""

import numpy as np

import concourse.bass as bass
import concourse.tile as tile
from concourse import bacc, mybir
from concourse.bass_utils import run_bass_kernel_spmd

N = 1048576
FEA = 128
N0 = 16384
NCORES = 8
W = 64                      # segments per group (PSUM free dim / one-hot width)
SEGS_PER_CORE = N0 // NCORES  # 2048
G = SEGS_PER_CORE // W      # 32 groups per core
P = 128

_prog_cache: dict = {}


def build_program(T: int, repeat: int = 1):
    """Build the SPMD Tile program for T atom-tiles (T*128 atoms) per group."""
    key = (T, repeat)
    if key in _prog_cache:
        return _prog_cache[key]

    f32 = mybir.dt.float32
    nc = bacc.Bacc("TRN2", target_bir_lowering=False, debug=False,
                   num_devices=NCORES)
    fea = nc.dram_tensor("fea", [G, P, T * P], f32, kind="ExternalInput").ap()
    idsr = nc.dram_tensor("idsr", [G, P, T], f32, kind="ExternalInput").ap()
    invc = nc.dram_tensor("invc", [G, P, W], f32, kind="ExternalInput").ap()
    out = nc.dram_tensor("out", [G, P, W], f32, kind="ExternalOutput").ap()

    # DMA chunk boundaries along the free dim of a group's fea block (engage
    # several DMA queues per group; each chunk is >=2KB per partition line).
    ncols = T * P
    nchunks = 8
    edges = [round(i * ncols / nchunks / 16) * 16 for i in range(nchunks + 1)]
    edges[-1] = ncols

    with tile.TileContext(nc) as tc:
        with (
            tc.tile_pool(name="const", bufs=1) as const_pool,
            tc.tile_pool(name="fea", bufs=3) as fea_pool,
            tc.tile_pool(name="meta", bufs=3) as meta_pool,
            tc.tile_pool(name="oh", bufs=4) as oh_pool,
            tc.tile_pool(name="evict", bufs=2) as evict_pool,
            tc.tile_pool(name="psum", bufs=2, space="PSUM") as psum_pool,
        ):
            iota_row = const_pool.tile([P, W], f32)
            nc.gpsimd.iota(iota_row[:], pattern=[[1, W]], base=0,
                           channel_multiplier=0,
                           allow_small_or_imprecise_dtypes=True)
            for _ in range(repeat):
                for g in range(G):
                    ids_sb = meta_pool.tile([P, T], f32)
                    nc.sync.dma_start(ids_sb[:], idsr[g])
                    invc_sb = meta_pool.tile([P, W], f32)
                    nc.sync.dma_start(invc_sb[:], invc[g])
                    fea_sb = fea_pool.tile([P, ncols], f32)
                    for j in range(nchunks):
                        nc.sync.dma_start(fea_sb[:, edges[j]:edges[j + 1]],
                                          fea[g][:, edges[j]:edges[j + 1]])
                    psum = psum_pool.tile([P, W], f32)
                    for t in range(T):
                        oh = oh_pool.tile([P, W], f32)
                        nc.vector.tensor_scalar(
                            out=oh[:], in0=iota_row[:],
                            scalar1=ids_sb[:, t:t + 1], scalar2=None,
                            op0=mybir.AluOpType.is_equal)
                        nc.tensor.matmul(
                            out=psum[:], lhsT=fea_sb[:, t * P:(t + 1) * P],
                            rhs=oh[:], start=(t == 0), stop=(t == T - 1))
                    out_sb = evict_pool.tile([P, W], f32)
                    nc.vector.tensor_tensor(out=out_sb[:], in0=psum[:],
                                            in1=invc_sb[:],
                                            op=mybir.AluOpType.mult)
                    nc.sync.dma_start(out[g], out_sb[:])
    nc.compile()
    _prog_cache[key] = nc
    return nc


def prepare_inputs(atom_fea: np.ndarray, segment_ids: np.ndarray):
    """Shard + pad + layout inputs for the 8 cores. Returns (in_maps, T)."""
    atom_fea = np.ascontiguousarray(atom_fea, dtype=np.float32)
    segment_ids = np.ascontiguousarray(segment_ids, dtype=np.int32)

    counts = np.bincount(segment_ids, minlength=N0).astype(np.int64)
    inv_counts = (1.0 / np.maximum(counts, 1)).astype(np.float32)

    ngroups = N0 // W
    bounds = np.searchsorted(segment_ids, np.arange(0, N0 + 1, W))
    gsizes = np.diff(bounds)
    T = max(1, int(np.ceil(gsizes.max() / P)))

    in_maps = []
    for c in range(NCORES):
        fea_c = np.zeros((G, P, T * P), dtype=np.float32)
        ids_c = np.full((G, P, T), -1.0, dtype=np.float32)
        invc_c = np.empty((G, P, W), dtype=np.float32)
        for g in range(G):
            gidx = c * G + g
            lo, hi = bounds[gidx], bounds[gidx + 1]
            n = hi - lo
            blk = np.zeros((T * P, FEA), dtype=np.float32)
            blk[:n] = atom_fea[lo:hi]
            fea_c[g] = blk.reshape(T, P, FEA).transpose(1, 0, 2).reshape(P, T * P)
            idb = np.full(T * P, -1.0, dtype=np.float32)
            idb[:n] = (segment_ids[lo:hi] - W * gidx).astype(np.float32)
            ids_c[g] = idb.reshape(T, P).T
            invc_c[g] = np.broadcast_to(inv_counts[W * gidx:W * (gidx + 1)],
                                        (P, W))
        in_maps.append({"fea": fea_c, "idsr": ids_c, "invc": invc_c})
    return in_maps, T


def assemble_output(results) -> np.ndarray:
    """[ncores][G, 128 fea, W seg] -> (N0, FEA)."""
    stacked = np.stack([results[c]["out"] for c in range(NCORES)])
    return np.ascontiguousarray(
        stacked.transpose(0, 1, 3, 2).reshape(N0, FEA))


def kernel(atom_fea: np.ndarray, segment_ids: np.ndarray,
           num_crystals=N0) -> np.ndarray:
    assert int(num_crystals) == N0
    assert atom_fea.shape == (N, FEA)
    in_maps, T = prepare_inputs(atom_fea, segment_ids)
    nc = build_program(T)
    res = run_bass_kernel_spmd(nc, in_maps, list(range(NCORES)))
    return assemble_output(res.results)


# revision 4
# speedup vs baseline: 378.0835x; 378.0835x over previous
"""Segment-mean (CGCNN crystal pooling) Bass kernel for 8 Trainium2 NeuronCores.

Reference computes, for sorted segment_ids over 1M atoms with 128 features:
    out[s] = sum(atom_fea[segment_ids == s]) / max(count(s), 1)   for s in [0, 16384)

Strategy (data-parallel over crystals, no cross-device communication):
  - Core c owns segments [2048*c, 2048*(c+1)) = 32 groups of 64 segments.
  - Host pads each 64-segment group's atoms to a uniform budget B = T*128 and
    lays them out partition-major: fea[g] is [128, T*128] where column block t
    holds atom tile t ([128 atoms in partitions] x [128 features]).
  - Device, per group: accumulate PSUM[128 fea, 64 seg] over T matmuls
    (lhsT = fea tile [128 atoms, 128 fea], rhs = one-hot [128 atoms, 64 segs]).
    The one-hot is built on DVE with one tensor_scalar is_equal against a
    constant iota row, using the per-atom relative segment id as the
    per-partition scalar. Padding atoms carry id -1 (never matches) and zero
    features. Evict PSUM with one elementwise multiply by 1/count (broadcast
    across partitions), then DMA [128 fea, 64 seg] to DRAM.
  - Host reassembles: transpose each [fea, seg] group slab to [seg, fea].
"""

import contextlib

import numpy as np

import concourse.bass as bass
import concourse.tile as tile
from concourse import bacc, mybir
from concourse.bass_utils import run_bass_kernel_spmd

N = 1048576
FEA = 128
N0 = 16384
NCORES = 8
W = 64                      # segments per group (PSUM free dim / one-hot width)
SEGS_PER_CORE = N0 // NCORES  # 2048
G = SEGS_PER_CORE // W      # 32 groups per core
P = 128

_prog_cache: dict = {}


def build_program(T: int, loop_repeat: int = 1):
    """Build the SPMD Tile program for T atom-tiles (T*128 atoms) per group.

    loop_repeat > 1 wraps the whole body in a hardware For_i loop (used only
    for timing; program size stays constant)."""
    key = (T, loop_repeat)
    if key in _prog_cache:
        return _prog_cache[key]

    f32 = mybir.dt.float32
    nc = bacc.Bacc("TRN2", target_bir_lowering=False, debug=False,
                   num_devices=NCORES)
    fea = nc.dram_tensor("fea", [G, P, T * P], f32, kind="ExternalInput").ap()
    idsr = nc.dram_tensor("idsr", [G, P, T], f32, kind="ExternalInput").ap()
    invc = nc.dram_tensor("invc", [G, P, W], f32, kind="ExternalInput").ap()
    out = nc.dram_tensor("out", [G, P, W], f32, kind="ExternalOutput").ap()

    # DMA chunk boundaries along the free dim of a group's fea block (engage
    # several DMA queues per group; each chunk is >=2KB per partition line).
    ncols = T * P
    nchunks = 8
    edges = [round(i * ncols / nchunks / 16) * 16 for i in range(nchunks + 1)]
    edges[-1] = ncols

    with tile.TileContext(nc) as tc:
        with (
            tc.tile_pool(name="const", bufs=1) as const_pool,
            tc.tile_pool(name="fea", bufs=3) as fea_pool,
            tc.tile_pool(name="meta", bufs=3) as meta_pool,
            tc.tile_pool(name="oh", bufs=4) as oh_pool,
            tc.tile_pool(name="evict", bufs=2) as evict_pool,
            tc.tile_pool(name="psum", bufs=2, space="PSUM") as psum_pool,
        ):
            iota_row = const_pool.tile([P, W], f32)
            nc.gpsimd.iota(iota_row[:], pattern=[[1, W]], base=0,
                           channel_multiplier=0,
                           allow_small_or_imprecise_dtypes=True)

            loop_ctx = (tc.For_i(0, loop_repeat, 1) if loop_repeat > 1
                        else contextlib.nullcontext())
            with loop_ctx:
                for g in range(G):
                    ids_sb = meta_pool.tile([P, T], f32)
                    nc.sync.dma_start(ids_sb[:], idsr[g])
                    invc_sb = meta_pool.tile([P, W], f32)
                    nc.sync.dma_start(invc_sb[:], invc[g])
                    fea_sb = fea_pool.tile([P, ncols], f32)
                    for j in range(nchunks):
                        nc.sync.dma_start(fea_sb[:, edges[j]:edges[j + 1]],
                                          fea[g][:, edges[j]:edges[j + 1]])
                    psum = psum_pool.tile([P, W], f32)
                    for t in range(T):
                        oh = oh_pool.tile([P, W], f32)
                        nc.vector.tensor_scalar(
                            out=oh[:], in0=iota_row[:],
                            scalar1=ids_sb[:, t:t + 1], scalar2=None,
                            op0=mybir.AluOpType.is_equal)
                        nc.tensor.matmul(
                            out=psum[:], lhsT=fea_sb[:, t * P:(t + 1) * P],
                            rhs=oh[:], start=(t == 0), stop=(t == T - 1))
                    out_sb = evict_pool.tile([P, W], f32)
                    nc.vector.tensor_tensor(out=out_sb[:], in0=psum[:],
                                            in1=invc_sb[:],
                                            op=mybir.AluOpType.mult)
                    nc.sync.dma_start(out[g], out_sb[:])
    nc.compile()
    _prog_cache[key] = nc
    return nc


def prepare_inputs(atom_fea: np.ndarray, segment_ids: np.ndarray):
    """Shard + pad + layout inputs for the 8 cores. Returns (in_maps, T)."""
    atom_fea = np.ascontiguousarray(atom_fea, dtype=np.float32)
    segment_ids = np.ascontiguousarray(segment_ids, dtype=np.int32)

    counts = np.bincount(segment_ids, minlength=N0).astype(np.int64)
    inv_counts = (1.0 / np.maximum(counts, 1)).astype(np.float32)

    bounds = np.searchsorted(segment_ids, np.arange(0, N0 + 1, W))
    gsizes = np.diff(bounds)
    T = max(1, int(np.ceil(gsizes.max() / P)))

    in_maps = []
    for c in range(NCORES):
        fea_c = np.zeros((G, P, T * P), dtype=np.float32)
        ids_c = np.full((G, P, T), -1.0, dtype=np.float32)
        invc_c = np.empty((G, P, W), dtype=np.float32)
        for g in range(G):
            gidx = c * G + g
            lo, hi = bounds[gidx], bounds[gidx + 1]
            n = hi - lo
            blk = np.zeros((T * P, FEA), dtype=np.float32)
            blk[:n] = atom_fea[lo:hi]
            fea_c[g] = blk.reshape(T, P, FEA).transpose(1, 0, 2).reshape(P, T * P)
            idb = np.full(T * P, -1.0, dtype=np.float32)
            idb[:n] = (segment_ids[lo:hi] - W * gidx).astype(np.float32)
            ids_c[g] = idb.reshape(T, P).T
            invc_c[g] = np.broadcast_to(inv_counts[W * gidx:W * (gidx + 1)],
                                        (P, W))
        in_maps.append({"fea": fea_c, "idsr": ids_c, "invc": invc_c})
    return in_maps, T


def assemble_output(results) -> np.ndarray:
    """[ncores][G, 128 fea, W seg] -> (N0, FEA)."""
    stacked = np.stack([results[c]["out"] for c in range(NCORES)])
    return np.ascontiguousarray(
        stacked.transpose(0, 1, 3, 2).reshape(N0, FEA))


def kernel(atom_fea: np.ndarray, segment_ids: np.ndarray,
           num_crystals=N0) -> np.ndarray:
    assert int(num_crystals) == N0
    assert atom_fea.shape == (N, FEA)
    in_maps, T = prepare_inputs(atom_fea, segment_ids)
    nc = build_program(T)
    res = run_bass_kernel_spmd(nc, in_maps, list(range(NCORES)))
    return assemble_output(res.results)


# revision 6
# speedup vs baseline: 1350.2139x; 3.5712x over previous
"""Segment-mean (CGCNN crystal pooling) Bass kernel for 8 Trainium2 NeuronCores.

Reference computes, for sorted segment_ids over 1M atoms with 128 features:
    out[s] = sum(atom_fea[segment_ids == s]) / max(count(s), 1)   s in [0, 16384)

Strategy (data-parallel over crystals, no cross-device communication):
  - Core c owns segments [2048*c, 2048*(c+1)) = 16 groups of W=128 segments.
  - Host pads each group's atoms to a uniform budget T*128 and lays them out
    partition-major: column block t of fea[g] ([128, T*128]) holds atom tile t
    ([128 atoms in partitions] x [128 features]).
  - Features ship as an exact bf16 hi/lo pair (hi = bf16(x), lo = bf16(x-hi)),
    so the bf16 matmul path (1 cycle/row + fast weight load) can be used while
    keeping ~2^-17 relative accuracy: one-hot entries are 0/1, so every
    product is exact and only the hi/lo split rounds.
  - Device, per group: ONE DVE tensor_tensor(is_equal) builds the whole
    group's one-hot block [128 atoms, T*W] from a tiled iota constant and a
    stride-0 broadcast of the per-atom relative segment ids (padding atoms
    carry id -1 and zero features). Then 2 bf16 matmuls per atom tile
    (lhsT = hi/lo tile [128 atoms, 128 fea], rhs = one-hot slice [128, W])
    accumulate into PSUM [128 fea, W segs]. Evict with one multiply by
    1/count (broadcast across partitions) and DMA out.
  - Host reassembles: transpose each [fea, seg] group slab to [seg, fea].

Measured on trn2 (8 cores, axon): ~224 us/kernel vs ~204 us pure-DMA floor
(input stream is 512MB + 3.1% padding at ~350 GB/s/core). Max relative error
vs the f32 reference: 2.6e-06.
"""

import contextlib

import ml_dtypes
import numpy as np

import concourse.bass as bass
import concourse.tile as tile
from concourse import bacc, mybir
from concourse.bass_utils import run_bass_kernel_spmd

N = 1048576
FEA = 128
N0 = 16384
NCORES = 8
W = 128                     # segments per group (PSUM free dim / one-hot width)
SEGS_PER_CORE = N0 // NCORES  # 2048
G = SEGS_PER_CORE // W      # 16 groups per core
P = 128
SB = 22                     # atom tiles per fea DMA block
FEA_BUFS = 6
BF16 = ml_dtypes.bfloat16

_prog_cache: dict = {}


def build_program(T: int, loop_repeat: int = 1):
    """SPMD Tile program for T atom-tiles (T*128 atoms) per group.

    loop_repeat > 1 wraps the body in a hardware For_i loop (timing only;
    program size stays constant)."""
    key = (T, loop_repeat)
    if key in _prog_cache:
        return _prog_cache[key]

    f32 = mybir.dt.float32
    bf16 = mybir.dt.bfloat16
    nc = bacc.Bacc("TRN2", target_bir_lowering=False, debug=False,
                   num_devices=NCORES)
    fhi = nc.dram_tensor("fhi", [G, P, T * P], bf16, kind="ExternalInput").ap()
    flo = nc.dram_tensor("flo", [G, P, T * P], bf16, kind="ExternalInput").ap()
    idsr = nc.dram_tensor("idsr", [G, P, T], bf16, kind="ExternalInput").ap()
    invc = nc.dram_tensor("invc", [G, P, W], f32, kind="ExternalInput").ap()
    out = nc.dram_tensor("out", [G, P, W], f32, kind="ExternalOutput").ap()

    sb = min(T, SB)
    blocks = [(s, min(s + sb, T)) for s in range(0, T, sb)]

    with tile.TileContext(nc) as tc:
        with (
            tc.tile_pool(name="const", bufs=1) as const_pool,
            tc.tile_pool(name="fea", bufs=FEA_BUFS) as fea_pool,
            tc.tile_pool(name="meta", bufs=3) as meta_pool,
            tc.tile_pool(name="oh", bufs=2) as oh_pool,
            tc.tile_pool(name="evict", bufs=2) as evict_pool,
            tc.tile_pool(name="psum", bufs=2, space="PSUM") as psum_pool,
        ):
            # constant [128, T*W] block where column block t holds 0..W-1
            # (per-slice iota: a step-0 iota pattern crashes the HW)
            iota_rep = const_pool.tile([P, T * W], bf16)
            for t in range(T):
                nc.gpsimd.iota(iota_rep[:, t * W:(t + 1) * W],
                               pattern=[[1, W]], base=0,
                               channel_multiplier=0,
                               allow_small_or_imprecise_dtypes=True)

            loop_ctx = (tc.For_i(0, loop_repeat, 1) if loop_repeat > 1
                        else contextlib.nullcontext())
            with loop_ctx:
                for g in range(G):
                    # meta on the ACT HWDGE queue, bulk fea on the sync queue
                    ids_sb = meta_pool.tile([P, T], bf16)
                    nc.scalar.dma_start(ids_sb[:], idsr[g])
                    invc_sb = meta_pool.tile([P, W], f32)
                    nc.scalar.dma_start(invc_sb[:], invc[g])
                    oh_blk = oh_pool.tile([P, T * W], bf16)
                    nc.vector.tensor_tensor(
                        out=oh_blk[:], in0=iota_rep[:],
                        in1=ids_sb[:].to_broadcast([P, T, W]),
                        op=mybir.AluOpType.is_equal)
                    psum = psum_pool.tile([P, W], f32)
                    for s, e in blocks:
                        hi_sb = fea_pool.tile([P, sb * P], bf16, tag="hi")
                        nc.sync.dma_start(hi_sb[:, :(e - s) * P],
                                          fhi[g][:, s * P:e * P])
                        lo_sb = fea_pool.tile([P, sb * P], bf16, tag="lo")
                        nc.sync.dma_start(lo_sb[:, :(e - s) * P],
                                          flo[g][:, s * P:e * P])
                        for t in range(s, e):
                            c0 = (t - s) * P
                            nc.tensor.matmul(
                                out=psum[:], lhsT=hi_sb[:, c0:c0 + P],
                                rhs=oh_blk[:, t * W:(t + 1) * W],
                                start=(t == 0), stop=False)
                            nc.tensor.matmul(
                                out=psum[:], lhsT=lo_sb[:, c0:c0 + P],
                                rhs=oh_blk[:, t * W:(t + 1) * W],
                                start=False, stop=(t == T - 1))
                    out_sb = evict_pool.tile([P, W], f32)
                    nc.vector.tensor_tensor(out=out_sb[:], in0=psum[:],
                                            in1=invc_sb[:],
                                            op=mybir.AluOpType.mult)
                    nc.scalar.dma_start(out[g], out_sb[:])
    nc.compile()
    _prog_cache[key] = nc
    return nc


def prepare_inputs(atom_fea: np.ndarray, segment_ids: np.ndarray):
    """Shard + pad + layout inputs for the 8 cores. Returns (in_maps, T)."""
    atom_fea = np.ascontiguousarray(atom_fea, dtype=np.float32)
    segment_ids = np.ascontiguousarray(segment_ids, dtype=np.int32)

    counts = np.bincount(segment_ids, minlength=N0).astype(np.int64)
    inv_counts = (1.0 / np.maximum(counts, 1)).astype(np.float32)

    bounds = np.searchsorted(segment_ids, np.arange(0, N0 + 1, W))
    T = max(1, int(np.ceil(np.diff(bounds).max() / P)))

    hi_full = atom_fea.astype(BF16)
    lo_full = (atom_fea - hi_full.astype(np.float32)).astype(BF16)

    in_maps = []
    for c in range(NCORES):
        hi_c = np.zeros((G, P, T * P), dtype=BF16)
        lo_c = np.zeros((G, P, T * P), dtype=BF16)
        ids_c = np.full((G, P, T), -1.0, dtype=BF16)
        invc_c = np.empty((G, P, W), dtype=np.float32)
        for g in range(G):
            gidx = c * G + g
            lo_i, hi_i = bounds[gidx], bounds[gidx + 1]
            n = hi_i - lo_i
            for dst, src in ((hi_c, hi_full), (lo_c, lo_full)):
                blk = np.zeros((T * P, FEA), dtype=BF16)
                blk[:n] = src[lo_i:hi_i]
                dst[g] = blk.reshape(T, P, FEA).transpose(1, 0, 2).reshape(
                    P, T * P)
            idb = np.full(T * P, -1.0, dtype=np.float32)
            idb[:n] = (segment_ids[lo_i:hi_i] - W * gidx).astype(np.float32)
            ids_c[g] = idb.reshape(T, P).T.astype(BF16)
            invc_c[g] = np.broadcast_to(inv_counts[W * gidx:W * (gidx + 1)],
                                        (P, W))
        in_maps.append({"fhi": hi_c, "flo": lo_c, "idsr": ids_c,
                        "invc": invc_c})
    return in_maps, T


def assemble_output(results) -> np.ndarray:
    """[ncores][G, 128 fea, W seg] -> (N0, FEA)."""
    stacked = np.stack([results[c]["out"] for c in range(NCORES)])
    return np.ascontiguousarray(
        stacked.transpose(0, 1, 3, 2).reshape(N0, FEA))


def kernel(atom_fea: np.ndarray, segment_ids: np.ndarray,
           num_crystals=N0) -> np.ndarray:
    assert int(num_crystals) == N0
    assert atom_fea.shape == (N, FEA)
    in_maps, T = prepare_inputs(atom_fea, segment_ids)
    nc = build_program(T)
    res = run_bass_kernel_spmd(nc, in_maps, list(range(NCORES)))
    return assemble_output(res.results)


# revision 8
# speedup vs baseline: 1381.0338x; 1.0228x over previous
"""Segment-mean (CGCNN crystal pooling) Bass kernel for 8 Trainium2 NeuronCores.

Reference computes, for sorted segment_ids over 1M atoms with 128 features:
    out[s] = sum(atom_fea[segment_ids == s]) / max(count(s), 1)   s in [0, 16384)

Strategy (data-parallel over crystals, no cross-device communication):
  - Core c owns segments [2048*c, 2048*(c+1)) = 16 groups of W=128 segments.
  - Host pads each group's atoms to a uniform budget T*128 and lays them out
    partition-major: column block t of fea[g] ([128, T*128]) holds atom tile t
    ([128 atoms in partitions] x [128 features]).
  - Features ship as an exact bf16 hi/lo pair (hi = bf16(x), lo = bf16(x-hi)),
    so the bf16 matmul path (1 cycle/row + fast weight load) can be used while
    keeping ~2^-17 relative accuracy: one-hot entries are 0/1, so every
    product is exact and only the hi/lo split rounds.
  - Device, per group: ONE DVE tensor_tensor(is_equal) builds the whole
    group's one-hot block [128 atoms, T*W] from a tiled iota constant and a
    stride-0 broadcast of the per-atom relative segment ids (padding atoms
    carry id -1 and zero features). Then 2 bf16 matmuls per atom tile
    (lhsT = hi/lo tile [128 atoms, 128 fea], rhs = one-hot slice [128, W])
    accumulate into PSUM [128 fea, W segs]. Evict with one multiply by
    1/count (broadcast across partitions) and DMA out.
  - Host reassembles: transpose each [fea, seg] group slab to [seg, fea].

Measured on trn2 (8 cores, axon): ~224 us/kernel vs ~204 us pure-DMA floor
(input stream is 512MB + 3.1% padding at ~350 GB/s/core). Max relative error
vs the f32 reference: 2.6e-06.
"""

import contextlib

import ml_dtypes
import numpy as np

import concourse.bass as bass
import concourse.tile as tile
from concourse import bacc, mybir
from concourse.bass_utils import run_bass_kernel_spmd

try:
    import jax
    from jax.experimental.shard_map import shard_map
    from jax.sharding import Mesh, NamedSharding, PartitionSpec
    from concourse.bass2jax import (_bass_exec_p, install_neuronx_cc_hook,
                                    partition_id_tensor)
    _HAVE_FAST_PATH = True
except Exception:  # pragma: no cover - fall back to run_bass_kernel_spmd
    _HAVE_FAST_PATH = False

N = 1048576
FEA = 128
N0 = 16384
NCORES = 8
W = 128                     # segments per group (PSUM free dim / one-hot width)
SEGS_PER_CORE = N0 // NCORES  # 2048
G = SEGS_PER_CORE // W      # 16 groups per core
P = 128
SB = 22                     # atom tiles per fea DMA block
FEA_BUFS = 6
BF16 = ml_dtypes.bfloat16

_prog_cache: dict = {}


def build_program(T: int, loop_repeat: int = 1):
    """SPMD Tile program for T atom-tiles (T*128 atoms) per group.

    loop_repeat > 1 wraps the body in a hardware For_i loop (timing only;
    program size stays constant)."""
    key = (T, loop_repeat)
    if key in _prog_cache:
        return _prog_cache[key]

    f32 = mybir.dt.float32
    bf16 = mybir.dt.bfloat16
    nc = bacc.Bacc("TRN2", target_bir_lowering=False, debug=False,
                   num_devices=NCORES)
    fhi = nc.dram_tensor("fhi", [G, P, T * P], bf16, kind="ExternalInput").ap()
    flo = nc.dram_tensor("flo", [G, P, T * P], bf16, kind="ExternalInput").ap()
    idsr = nc.dram_tensor("idsr", [G, P, T], bf16, kind="ExternalInput").ap()
    invc = nc.dram_tensor("invc", [G, P, W], f32, kind="ExternalInput").ap()
    out = nc.dram_tensor("out", [G, P, W], f32, kind="ExternalOutput").ap()

    sb = min(T, SB)
    blocks = [(s, min(s + sb, T)) for s in range(0, T, sb)]

    with tile.TileContext(nc) as tc:
        with (
            tc.tile_pool(name="const", bufs=1) as const_pool,
            tc.tile_pool(name="fea", bufs=FEA_BUFS) as fea_pool,
            tc.tile_pool(name="meta", bufs=3) as meta_pool,
            tc.tile_pool(name="oh", bufs=2) as oh_pool,
            tc.tile_pool(name="evict", bufs=2) as evict_pool,
            tc.tile_pool(name="psum", bufs=2, space="PSUM") as psum_pool,
        ):
            # constant [128, T*W] block where column block t holds 0..W-1
            # (per-slice iota: a step-0 iota pattern crashes the HW)
            iota_rep = const_pool.tile([P, T * W], bf16)
            for t in range(T):
                nc.gpsimd.iota(iota_rep[:, t * W:(t + 1) * W],
                               pattern=[[1, W]], base=0,
                               channel_multiplier=0,
                               allow_small_or_imprecise_dtypes=True)

            loop_ctx = (tc.For_i(0, loop_repeat, 1) if loop_repeat > 1
                        else contextlib.nullcontext())
            with loop_ctx:
                for g in range(G):
                    # meta on the ACT HWDGE queue, bulk fea on the sync queue
                    ids_sb = meta_pool.tile([P, T], bf16)
                    nc.scalar.dma_start(ids_sb[:], idsr[g])
                    invc_sb = meta_pool.tile([P, W], f32)
                    nc.scalar.dma_start(invc_sb[:], invc[g])
                    oh_blk = oh_pool.tile([P, T * W], bf16)
                    nc.vector.tensor_tensor(
                        out=oh_blk[:], in0=iota_rep[:],
                        in1=ids_sb[:].to_broadcast([P, T, W]),
                        op=mybir.AluOpType.is_equal)
                    psum = psum_pool.tile([P, W], f32)
                    for s, e in blocks:
                        hi_sb = fea_pool.tile([P, sb * P], bf16, tag="hi")
                        nc.sync.dma_start(hi_sb[:, :(e - s) * P],
                                          fhi[g][:, s * P:e * P])
                        lo_sb = fea_pool.tile([P, sb * P], bf16, tag="lo")
                        nc.sync.dma_start(lo_sb[:, :(e - s) * P],
                                          flo[g][:, s * P:e * P])
                        for t in range(s, e):
                            c0 = (t - s) * P
                            nc.tensor.matmul(
                                out=psum[:], lhsT=hi_sb[:, c0:c0 + P],
                                rhs=oh_blk[:, t * W:(t + 1) * W],
                                start=(t == 0), stop=False)
                            nc.tensor.matmul(
                                out=psum[:], lhsT=lo_sb[:, c0:c0 + P],
                                rhs=oh_blk[:, t * W:(t + 1) * W],
                                start=False, stop=(t == T - 1))
                    out_sb = evict_pool.tile([P, W], f32)
                    nc.vector.tensor_tensor(out=out_sb[:], in0=psum[:],
                                            in1=invc_sb[:],
                                            op=mybir.AluOpType.mult)
                    nc.scalar.dma_start(out[g], out_sb[:])
    nc.compile()
    _prog_cache[key] = nc
    return nc


def prepare_inputs(atom_fea: np.ndarray, segment_ids: np.ndarray):
    """Shard + pad + layout inputs for the 8 cores. Returns (in_maps, T)."""
    atom_fea = np.ascontiguousarray(atom_fea, dtype=np.float32)
    segment_ids = np.ascontiguousarray(segment_ids, dtype=np.int32)

    counts = np.bincount(segment_ids, minlength=N0).astype(np.int64)
    inv_counts = (1.0 / np.maximum(counts, 1)).astype(np.float32)

    bounds = np.searchsorted(segment_ids, np.arange(0, N0 + 1, W))
    T = max(1, int(np.ceil(np.diff(bounds).max() / P)))

    hi_full = atom_fea.astype(BF16)
    lo_full = (atom_fea - hi_full.astype(np.float32)).astype(BF16)

    in_maps = []
    for c in range(NCORES):
        hi_c = np.zeros((G, P, T * P), dtype=BF16)
        lo_c = np.zeros((G, P, T * P), dtype=BF16)
        ids_c = np.full((G, P, T), -1.0, dtype=BF16)
        invc_c = np.empty((G, P, W), dtype=np.float32)
        for g in range(G):
            gidx = c * G + g
            lo_i, hi_i = bounds[gidx], bounds[gidx + 1]
            n = hi_i - lo_i
            for dst, src in ((hi_c, hi_full), (lo_c, lo_full)):
                blk = np.zeros((T * P, FEA), dtype=BF16)
                blk[:n] = src[lo_i:hi_i]
                dst[g] = blk.reshape(T, P, FEA).transpose(1, 0, 2).reshape(
                    P, T * P)
            idb = np.full(T * P, -1.0, dtype=np.float32)
            idb[:n] = (segment_ids[lo_i:hi_i] - W * gidx).astype(np.float32)
            ids_c[g] = idb.reshape(T, P).T.astype(BF16)
            invc_c[g] = np.broadcast_to(inv_counts[W * gidx:W * (gidx + 1)],
                                        (P, W))
        in_maps.append({"fhi": hi_c, "flo": lo_c, "idsr": ids_c,
                        "invc": invc_c})
    return in_maps, T


def assemble_output(results) -> np.ndarray:
    """[ncores][G, 128 fea, W seg] -> (N0, FEA)."""
    stacked = np.stack([results[c]["out"] for c in range(NCORES)])
    return np.ascontiguousarray(
        stacked.transpose(0, 1, 3, 2).reshape(N0, FEA))


def _run_spmd_fast(nc, in_maps):
    """Execute the SPMD program on cores 0-7 via PJRT with explicit sharded
    device_put (same _bass_exec_p mechanism run_bass_kernel_spmd uses under
    axon, minus its per-call retrace and slow implicit transfers)."""
    install_neuronx_cc_hook()
    partition_name = (nc.partition_id_tensor.name
                      if nc.partition_id_tensor else None)
    in_names, out_names, out_avals = [], [], []
    for alloc in nc.m.functions[0].allocations:
        if not isinstance(alloc, mybir.MemoryLocationSet):
            continue
        name = alloc.memorylocations[0].name
        if alloc.kind == "ExternalInput":
            if name != partition_name:
                in_names.append(name)
        elif alloc.kind == "ExternalOutput":
            out_names.append(name)
            out_avals.append(jax.core.ShapedArray(
                tuple(alloc.tensor_shape), mybir.dt.np(alloc.dtype)))
    n_params = len(in_names)
    all_in_names = list(in_names) + list(out_names)
    if partition_name is not None:
        all_in_names.append(partition_name)

    def _body(*args):
        operands = list(args)
        if partition_name is not None:
            operands.append(partition_id_tensor())
        return tuple(_bass_exec_p.bind(
            *operands, out_avals=tuple(out_avals),
            in_names=tuple(all_in_names), out_names=tuple(out_names),
            lowering_input_output_aliases=(), sim_require_finite=True,
            sim_require_nnan=True, nc=nc))

    devices = jax.devices()[:NCORES]
    assert len(devices) == NCORES, f"need {NCORES} devices, got {devices}"
    mesh = Mesh(np.asarray(devices), ("core",))
    spec = PartitionSpec("core")
    fn = jax.jit(
        shard_map(_body, mesh=mesh, in_specs=(spec,) * (n_params + len(out_names)),
                  out_specs=(spec,) * len(out_names), check_rep=False),
        keep_unused=True)
    sh = NamedSharding(mesh, spec)
    dev_in = [
        jax.device_put(
            np.concatenate([np.asarray(in_maps[c][name])
                            for c in range(NCORES)], axis=0), sh)
        for name in in_names
    ] + [
        jax.device_put(
            np.zeros((NCORES * a.shape[0], *a.shape[1:]), a.dtype), sh)
        for a in out_avals
    ]
    outs = fn(*dev_in)
    jax.block_until_ready(outs)
    return [
        {name: np.asarray(outs[i]).reshape(NCORES, *out_avals[i].shape)[c]
         for i, name in enumerate(out_names)}
        for c in range(NCORES)
    ]


def kernel(atom_fea: np.ndarray, segment_ids: np.ndarray,
           num_crystals=N0) -> np.ndarray:
    assert int(num_crystals) == N0
    assert atom_fea.shape == (N, FEA)
    in_maps, T = prepare_inputs(atom_fea, segment_ids)
    nc = build_program(T)
    if _HAVE_FAST_PATH:
        try:
            return assemble_output(_run_spmd_fast(nc, in_maps))
        except Exception:
            pass
    res = run_bass_kernel_spmd(nc, in_maps, list(range(NCORES)))
    return assemble_output(res.results)
